# revision 33
# baseline (speedup 1.0000x reference)
"""Causal attention (B=4, T=2048, D=1024, H=16) on 8 TRN2 NeuronCores.

Sharding: core c handles batch b = c//2 and head-group hg = c%2 (8 heads).
Each core computes qkv = x_b @ Wqkv_shard, flash-style causal attention for
its 8 heads, and a partial output projection y_heads @ Wproj_rows.  The two
cores of a batch produce partial sums over the projection's contraction dim;
the host adds them (the tensor-parallel all-reduce done at unshard time).

Layouts on device (per core):
  x^T   [1024c, 2048t]  bf16  - via f32->bf16 cast + DMA xbar transpose
  Q^T,K^T [64d, t] per head packed as [128, cc, 512] tiles (head pair on
        partitions 0-63 / 64-127) - computed with W chunks stationary
  V     [t, 512] natural - computed with x^T chunks stationary
  S^T   [k=128, q<=512] in PSUM - 2-head row-packed matmuls (d=64 contraction)
  A^T   = exp(S^T/8) bf16 in SBUF (no-max softmax; scores are O(4))
  O^T   [d=128(2 heads), q=512] - col-packed A^T@V + ones-matmul rowsums,
        normalized by 1/rowsum on DVE
  out   [t=128, o=512] psum tiles - O^T chunks stationary vs Wproj
"""
import sys

if "/opt/trn_rl_repo" not in sys.path:
    sys.path.insert(0, "/opt/trn_rl_repo")

import numpy as np

import concourse.bass as bass
import concourse.mybir as mybir
import concourse.tile as tile
from concourse.bass_utils import run_bass_kernel_spmd
from concourse.vector_clock import ScopedClock

F32 = mybir.dt.float32
BF16 = mybir.dt.bfloat16
AF = mybir.ActivationFunctionType

T = 2048
D = 1024
NH_LOC = 8          # heads per core
HD = 64             # head dim
NT = 4              # 512-wide t/q superblocks
NC_ = 8             # c-chunks of 128
NPAIR = 4           # head pairs per core

# ---------------------------------------------------------------------------
# Walrus rejects >1 sync wait on a Drain; spill excess waits onto extra
# drains issued right after (same engine, still before the sem clear).
_MAXW = 1


def _patched_drain_and_barrier(self, tick_clock, wait_clock):
    drain_inst = self.nc.sync.drain()
    wait_clock.add_sem_waits(
        drain_inst.ins, ScopedClock({None: tick_clock.global_clock})
    )
    waits = list(drain_inst.ins.sync_info.on_wait or [])
    if len(waits) > _MAXW:
        drain_inst.ins.sync_info.on_wait = waits[:_MAXW]
        rest = waits[_MAXW:]
        for i in range(0, len(rest), _MAXW):
            extra = self.nc.sync.drain()
            extra.ins.sync_info = mybir.SyncInfo(
                on_wait=rest[i : i + _MAXW], on_update=[]
            )

    self.nc.all_engine_barrier()
    assert self.sems is not None
    popped = self.nc._tile_sem_poison_stack.pop()
    assert popped is self._sem_poison
    self.nc.clear_and_free_semaphores(list(self.sems.allocated().values()))
    self.nc.all_engine_barrier()


tile.TileContext._drain_and_barrier = _patched_drain_and_barrier


def _split_multi_waits(nc, max_waits=1):
    """This container's walrus accepts at most one sync wait per instruction.

    Insert same-engine `nop` carrier instructions immediately before any
    instruction holding >1 waits, moving the excess waits onto them.  The
    engine executes in order, so this is semantics-preserving.
    """
    fn = nc.m.functions[0]
    snapshots = [(blk, list(blk.instructions)) for blk in fn.blocks]
    rebuilt = []
    for blk, insts in snapshots:
        new_list = []
        for inst in insts:
            si = inst.sync_info
            waits = list(si.on_wait) if si and si.on_wait else []
            if len(waits) > max_waits:
                extra, keep = waits[:-max_waits], waits[-max_waits:]
                for j in range(0, len(extra), max_waits):
                    c = nc.engines[inst.engine].nop(nofuse=True)
                    c.ins.sync_info = mybir.SyncInfo(
                        on_wait=extra[j : j + max_waits], on_update=[]
                    )
                    new_list.append(c.ins)
                si.on_wait = keep
            new_list.append(inst)
        rebuilt.append((blk, new_list))
    for blk, new_list in rebuilt:
        blk.instructions = new_list
# ---------------------------------------------------------------------------


def build_nc():
    nc = bass.Bass("TRN2", target_bir_lowering=False, debug=False)
    x = nc.dram_tensor("x", [T, D], F32, kind="ExternalInput").ap()
    wqkv = nc.dram_tensor("wqkv", [D, 1536], F32, kind="ExternalInput").ap()
    wproj = nc.dram_tensor("wproj", [512, D], F32, kind="ExternalInput").ap()
    masks = nc.dram_tensor("masks", [4, 128, 512], F32, kind="ExternalInput").ap()
    out = nc.dram_tensor("out", [T, D], F32, kind="ExternalOutput").ap()

    wproj3 = wproj.rearrange("(po pi) o -> pi po o", pi=128)  # [128, 4, 1024]
    wqkv3 = wqkv.rearrange("(po pi) o -> pi po o", pi=128)    # [128, 8, 1536]

    with tile.TileContext(nc) as tc:
        with (
            tc.tile_pool(name="persist", bufs=1) as persist,
            tc.tile_pool(name="work", bufs=4) as work,
            tc.tile_pool(name="xtp", bufs=2) as xtp,
            tc.tile_pool(name="xfp", bufs=3) as xfp,
            tc.tile_pool(name="atp", bufs=12) as atp,
            tc.tile_pool(name="rrp", bufs=6) as rrp,
            tc.tile_pool(name="pss", bufs=2, space="PSUM") as pss,
            tc.tile_pool(name="psacc", bufs=2, space="PSUM") as psacc,
            tc.tile_pool(name="dram", bufs=1, space="DRAM") as dram,
        ):
            def emit_prep(i):
                # x slice -> bf16 -> DRAM -> transposed load.  Stores go on
                # the ACT HWDGE queue so loads never queue behind them.
                for tt2 in range(4):
                    t0 = 512 * i + 128 * tt2
                    xf = xfp.tile([128, D], F32, tag="xf", name="xf")
                    nc.sync.dma_start(out=xf[:, :D], in_=x[t0 : t0 + 128, :])
                    xb = work.tile([128, D], BF16, tag="work_bf16", name="xb")
                    nc.vector.tensor_copy(out=xb, in_=xf[:, :D])
                    nc.scalar.dma_start(
                        out=xbf_dram[i][128 * tt2 : 128 * (tt2 + 1), :], in_=xb
                    )
                xT = xtp.tile([128, NC_, 512], BF16, tag="xT", name="xT")
                for j in range(NC_):
                    nc.sync.dma_start_transpose(
                        out=xT[:, j, :], in_=xbf_dram[i][:, 128 * j : 128 * (j + 1)]
                    )
                return xT

            # persistent per-superblock tensors
            qkvT = [
                persist.tile([128, NC_, 512], BF16, tag=f"qkvT{i}", name=f"qkvT{i}")
                for i in range(NT)
            ]
            v_c = [
                persist.tile([128, 512], BF16, tag=f"v{kk}", name=f"v{kk}") for kk in range(16)
            ]
            oT = [
                [persist.tile([128, 512], BF16, tag=f"oT{p}_{i}", name=f"oT{p}_{i}") for i in range(NT)]
                for p in range(NPAIR)
            ]
            xbf_dram = [dram.tile([512, D], BF16, tag=f"xbf{i}", name=f"xbf{i}") for i in range(NT)]
            xT_next = emit_prep(0)

            # ---- constants / weights ----------------------------------
            w_bf = persist.tile([128, NC_, 1536], BF16, tag="w_bf")
            for j in range(NC_):
                wf = work.tile([128, 1536], F32, tag="work_f32", name="wf")
                nc.sync.dma_start(out=wf, in_=wqkv3[:, j, :])
                nc.vector.tensor_copy(out=w_bf[:, j, :], in_=wf)
            wp_bf = persist.tile([128, NPAIR, 1024], BF16, tag="wp_bf")
            for p in range(NPAIR):
                wf = work.tile([128, 1536], F32, tag="work_f32")
                nc.sync.dma_start(out=wf[:, :1024], in_=wproj3[:, p, :])
                nc.vector.tensor_copy(out=wp_bf[:, p, :], in_=wf[:, :1024])
            mask_bf = persist.tile([128, 4, 512], BF16, tag="mask_bf")
            for dk in range(4):
                mf = work.tile([128, 1536], F32, tag="work_f32")
                nc.sync.dma_start(out=mf[:, :512], in_=masks[dk])
                nc.vector.tensor_copy(out=mask_bf[:, dk, :], in_=mf[:, :512])
            # ones [128, 64]: rowsum matmul replicates each head's row-sums
            # across 64 psum partitions, so normalize needs no broadcast AP
            ones = persist.tile([128, 64], BF16, tag="ones")
            nc.vector.memset(ones, 1.0)


            for i in range(NT):
                xT = xT_next
                # ---- QKV for this superblock --------------------------
                # Q^T,K^T: W chunks stationary, x^T moving
                for cc in range(8):
                    ps = psacc.tile([128, 1024], F32, tag="acc", name="ps")[:, :512]
                    for j in range(NC_):
                        nc.tensor.matmul(
                            ps,
                            lhsT=w_bf[:, j, 128 * cc : 128 * (cc + 1)],
                            rhs=xT[:, j, :],
                            start=(j == 0),
                            stop=(j == NC_ - 1),
                        )
                    nc.vector.tensor_copy(out=qkvT[i][:, cc, :], in_=ps)
                # V natural: x^T chunks stationary, W_v moving
                for tt2 in range(4):
                    ps = psacc.tile([128, 1024], F32, tag="acc", name="ps")[:, :512]
                    for j in range(NC_):
                        nc.tensor.matmul(
                            ps,
                            lhsT=xT[:, j, 128 * tt2 : 128 * (tt2 + 1)],
                            rhs=w_bf[:, j, 1024:1536],
                            start=(j == 0),
                            stop=(j == NC_ - 1),
                        )
                    nc.vector.tensor_copy(out=v_c[4 * i + tt2], in_=ps)
                if i + 1 < NT:
                    xT_next = emit_prep(i + 1)

                # ---- attention for q-superblock i ---------------------
                nkk = 4 * i + 4
                for p in range(NPAIR):
                    av_ps = psacc.tile([128, 1024], F32, tag="acc", name="av_ps")
                    rs_ps = psacc.tile([128, 1024], F32, tag="acc", name="rs_ps")
                    for g2 in range((nkk + 1) // 2):
                        kks = [kk for kk in (2 * g2, 2 * g2 + 1) if kk < nkk]
                        # causal trim: chunk kk only covers q >= 128*dk.
                        # Chunk sl=0 ends at col 512; sl=1 starts at 512, so
                        # packed regions stay contiguous (no uninit psum).
                        q0s = [128 * max(0, kk - 4 * i) for kk in kks]
                        cols = []  # (c0, c1) region in s_ps/aT per chunk
                        for sl, kk in enumerate(kks):
                            if sl == 0:
                                cols.append((q0s[0], 512))
                            else:
                                cols.append((512, 1024 - q0s[1]))
                        # head A (partitions/array rows 0-63) and head B
                        # (64-127) interleaved so their tile_position-packed
                        # matmuls sit adjacent in the PE queue and overlap
                        # in the array.
                        s_2 = {}
                        for base in (0, 64):
                            s_ps = pss.tile([128, 1024], F32, tag="ps", name="s_ps")
                            s_2[base] = s_ps
                            for sl, kk in enumerate(kks):
                                c0, c1 = cols[sl]
                                nc.tensor.matmul(
                                    s_ps[:, c0:c1],
                                    lhsT=qkvT[kk // 4][
                                        base : base + 64,
                                        4 + p,
                                        128 * (kk % 4) : 128 * (kk % 4 + 1),
                                    ],
                                    rhs=qkvT[i][base : base + 64, p, q0s[sl] :],
                                    start=True,
                                    stop=True,
                                    tile_position=(base, 0),
                                )
                        a_2 = {}
                        for base in (0, 64):
                            aT = atp.tile([128, 1024], BF16, tag="aT", name="aT")
                            a_2[base] = aT
                            # exp(S/8): 1/sqrt(64) folded into the ACT affine
                            nc.scalar.activation(
                                out=aT[:, cols[0][0] : cols[-1][1]],
                                in_=s_2[base][:, cols[0][0] : cols[-1][1]],
                                func=AF.Exp,
                                scale=0.125,
                            )
                            for sl, kk in enumerate(kks):
                                if kk - 4 * i >= 0:  # zero diagonal triangle
                                    c0 = cols[sl][0]
                                    nc.vector.tensor_mul(
                                        out=aT[:, c0 : c0 + 128],
                                        in0=aT[:, c0 : c0 + 128],
                                        in1=mask_bf[:, 0, :128],
                                    )
                        for sl, kk in enumerate(kks):
                            for base in (0, 64):
                                lh = 2 * p + (base // 64)
                                a_sl = a_2[base][:, cols[sl][0] : cols[sl][1]]
                                bo = 8 * base  # head B in its own bank
                                nc.tensor.matmul(
                                    av_ps[base : base + 64, bo + q0s[sl] : bo + 512],
                                    lhsT=v_c[kk][:, 64 * lh : 64 * (lh + 1)],
                                    rhs=a_sl,
                                    start=(kk == 0),
                                    stop=(kk == nkk - 1),
                                    tile_position=(0, base),
                                )
                            for base in (0, 64):
                                a_sl = a_2[base][:, cols[sl][0] : cols[sl][1]]
                                bo = 8 * base
                                nc.tensor.matmul(
                                    rs_ps[base : base + 64, bo + q0s[sl] : bo + 512],
                                    lhsT=ones,
                                    rhs=a_sl,
                                    start=(kk == 0),
                                    stop=(kk == nkk - 1),
                                    tile_position=(0, base),
                                )
                    # normalize: O^T row-scale by 1/rowsum (already replicated)
                    rr = rrp.tile([128, 512], F32, tag="rr")
                    nc.vector.reciprocal(out=rr[0:64, :], in_=rs_ps[0:64, 0:512])
                    nc.vector.reciprocal(out=rr[64:128, :], in_=rs_ps[64:128, 512:1024])
                    nc.vector.tensor_tensor(
                        oT[p][i][0:64, :], av_ps[0:64, 0:512], rr[0:64, :],
                        mybir.AluOpType.mult,
                    )
                    nc.vector.tensor_tensor(
                        oT[p][i][64:128, :], av_ps[64:128, 512:1024], rr[64:128, :],
                        mybir.AluOpType.mult,
                    )

                # ---- partial output projection for superblock i -------
                for tt2 in range(4):
                    t0 = 512 * i + 128 * tt2
                    for oo in range(2):
                        ps = psacc.tile([128, 1024], F32, tag="acc", name="ps")[:, :512]
                        for p in range(NPAIR):
                            nc.tensor.matmul(
                                ps,
                                lhsT=oT[p][i][:, 128 * tt2 : 128 * (tt2 + 1)],
                                rhs=wp_bf[:, p, 512 * oo : 512 * (oo + 1)],
                                start=(p == 0),
                                stop=(p == NPAIR - 1),
                            )
                        ostage = work.tile([128, 512], F32, tag="work_out")
                        nc.vector.tensor_copy(out=ostage, in_=ps)
                        nc.scalar.dma_start(
                            out=out[t0 : t0 + 128, 512 * oo : 512 * (oo + 1)],
                            in_=ostage,
                        )
    _split_multi_waits(nc)
    return nc


def _make_masks():
    p = np.arange(128)[:, None]
    f = np.arange(512)[None, :]
    return np.stack(
        [(f >= 128 * dk + p).astype(np.float32) for dk in range(4)]
    )  # [4, 128, 512]


def _in_maps(x, W_attn, W_proj):
    masks = _make_masks()
    maps = []
    for core in range(8):
        b, hg = core // 2, core % 2
        qc = slice(512 * hg, 512 * (hg + 1))
        kc = slice(1024 + 512 * hg, 1024 + 512 * (hg + 1))
        vc = slice(2048 + 512 * hg, 2048 + 512 * (hg + 1))
        wqkv = np.ascontiguousarray(
            np.concatenate([W_attn[:, qc], W_attn[:, kc], W_attn[:, vc]], axis=1)
        )
        wproj = np.ascontiguousarray(W_proj[512 * hg : 512 * (hg + 1), :])
        maps.append(
            {
                "x": np.ascontiguousarray(x[b]),
                "wqkv": wqkv,
                "wproj": wproj,
                "masks": masks,
            }
        )
    return maps


_RUNNER = None


def _make_runner():
    """Compile once; return run(in_maps) -> list[dict] executing on 8 cores.

    Mirrors bass2jax.run_bass_via_pjrt's multi-core branch but caches the
    jitted executable so repeated calls skip XLA/neuron recompilation.
    """
    import jax
    from jax.sharding import Mesh, PartitionSpec
    from concourse import bass2jax as b2j

    try:
        from jax.experimental.shard_map import shard_map
    except ImportError:  # newer jax
        from jax import shard_map

    nc = build_nc()
    b2j.install_neuronx_cc_hook()
    n_cores = 8

    partition_name = nc.partition_id_tensor.name if nc.partition_id_tensor else None
    in_names, out_names, out_avals, zero_shapes = [], [], [], []
    for alloc in nc.m.functions[0].allocations:
        if not isinstance(alloc, mybir.MemoryLocationSet):
            continue
        name = alloc.memorylocations[0].name
        if alloc.kind == "ExternalInput":
            if name != partition_name:
                in_names.append(name)
        elif alloc.kind == "ExternalOutput":
            out_names.append(name)
            shape = tuple(alloc.tensor_shape)
            dtype = mybir.dt.np(alloc.dtype)
            out_avals.append(jax.core.ShapedArray(shape, dtype))
            zero_shapes.append((shape, dtype))
    n_params = len(in_names)
    n_outs = len(out_avals)
    all_in_names = list(in_names) + list(out_names)
    if partition_name is not None:
        all_in_names.append(partition_name)
    donate = tuple(range(n_params, n_params + n_outs))

    def _body(*args):
        operands = list(args)
        if partition_name is not None:
            operands.append(b2j.partition_id_tensor())
        outs = b2j._bass_exec_p.bind(
            *operands,
            out_avals=tuple(out_avals),
            in_names=tuple(all_in_names),
            out_names=tuple(out_names),
            lowering_input_output_aliases=(),
            sim_require_finite=True,
            sim_require_nnan=True,
            nc=nc,
        )
        return tuple(outs)

    devices = jax.devices()[:n_cores]
    mesh = Mesh(np.asarray(devices), ("core",))
    in_specs = (PartitionSpec("core"),) * (n_params + n_outs)
    out_specs = (PartitionSpec("core"),) * n_outs
    sharded = jax.jit(
        shard_map(
            _body, mesh=mesh, in_specs=in_specs, out_specs=out_specs, check_rep=False
        ),
        donate_argnums=donate,
        keep_unused=True,
    )

    def run(in_maps):
        per_core = [[np.asarray(m[name]) for name in in_names] for m in in_maps]
        concat_in = [
            np.concatenate([per_core[c][i] for c in range(n_cores)], axis=0)
            for i in range(n_params)
        ]
        concat_zeros = [
            np.zeros((n_cores * s[0], *s[1:]), d) for (s, d) in zero_shapes
        ]
        out_arrs = sharded(*concat_in, *concat_zeros)
        out_arrs = [np.asarray(a) for a in out_arrs]
        return [
            {
                name: out_arrs[i].reshape(n_cores, *out_avals[i].shape)[c]
                for i, name in enumerate(out_names)
            }
            for c in range(n_cores)
        ]

    return run


def kernel(x, W_attn, W_proj):
    global _RUNNER
    x = np.asarray(x, dtype=np.float32)
    W_attn = np.asarray(W_attn, dtype=np.float32)
    W_proj = np.asarray(W_proj, dtype=np.float32)
    if _RUNNER is None:
        _RUNNER = _make_runner()
    results = _RUNNER(_in_maps(x, W_attn, W_proj))
    outs = [results[i]["out"] for i in range(8)]
    return np.stack([outs[2 * b] + outs[2 * b + 1] for b in range(4)]).astype(
        np.float32
    )
